# revision 3
# baseline (speedup 1.0000x reference)
"""LIF (leaky integrate-and-fire) forward kernel for Trainium2, 8-core SPMD.

Reference semantics (per element, scan over T):
    u = 0.5*u + x_t
    o_t = (u - 1 >= 0) ? 1.0 : 0.0
    u = u - o_t

Sharding: pure data parallel over batch B=32 -> 4 batches per core.
Per-core shard: x [4, 16, 128, 1024] f32; C=128 on the SBUF partition dim,
(b, h*w) on the free dim -> a [128, 4096] tile per timestep.

Work is split across three compute engines by free-dim column slices:

  D-slice [0, FD): state v (post-reset potential), all on DVE:
      u' = (v * 0.5) + x_t          stt   (DVE, 2-operand)
      o  = Sign(u' - 1) -> uint8    ACT   (saturating cast: -1 -> 0)
      v  = (o * -1.0) + u'          stt   (DVE, u8 operand casts for free)

  G-slices [FD, 4096) x3: state h = 0.5*v, integrate runs on GPSIMD
  (which only supports tensor_tensor add/sub, hence the halved state):
      u' = h + x_t                  tt    (GPSIMD)
      o  = Sign(u' - 1) -> uint8    ACT
      w  = (o * -1.0) + u'          stt   (DVE)
      h  = Copy(w * 0.5)            ACT   (exact: *0.5 and v-o are exact)

All f32 arithmetic rounds exactly like the jax reference; the only
divergence is u' == 1.0 exactly (Sign(0)=0 stores o=0 where the reference
fires) - measure-zero on randn inputs, far inside the 2e-2 gate.

Engine loads per step (measured per-op costs, [128, 4096] units):
  DVE ~6.3us (2 stt on D + 3 stt on G), GPSIMD ~6.3us (3 tt),
  ACT ~5.9us (4 Sign + 3 halving copies), in-DMA 2 MiB, out-DMA 0.5 MiB.

Raw bass (no TileContext): this walrus build caps embedded sync waits at 1
per DMA/DVE instruction; standalone wait_ge instructions have no limit, so
every dependency is a standalone wait against a per-engine monotonic
semaphore (s_v/s_g/s_a: +1 per compute instr; s_x/s_o: +16 per DMA).
"""

import numpy as np

B, T, C, HW = 32, 16, 128, 1024
NCORES = 8
BLOC = B // NCORES  # 4 batches per core
FREE = BLOC * HW    # 4096
NX = 3              # x_t buffer slots

# Column split: D handled entirely by DVE; G1-3 integrate on GPSIMD.
FD = 1728
GS = [790, 790, 788]          # G slice widths; FD + sum(GS) == FREE
GOFF = [FD, FD + GS[0], FD + GS[0] + GS[1]]
WG = sum(GS)

_cached = {}


def _build_nc():
    import concourse.bass as bass
    import concourse.mybir as mybir
    from contextlib import ExitStack

    f32 = mybir.dt.float32
    u8 = mybir.dt.uint8
    Alu = mybir.AluOpType
    Act = mybir.ActivationFunctionType

    nc = bass.Bass()
    x_d = nc.declare_dram_parameter("x", [BLOC, T, C, HW], f32, isOutput=False)
    o_d = nc.declare_dram_parameter("o", [BLOC, T, C, HW], u8, isOutput=True)

    with ExitStack() as ctx:
        xt = [
            ctx.enter_context(nc.sbuf_tensor(f"xt{i}", [C, FREE], f32))
            for i in range(NX)
        ]
        up = [
            ctx.enter_context(nc.sbuf_tensor(f"up{i}", [C, FREE], f32))
            for i in range(2)
        ]
        ot = [
            ctx.enter_context(nc.sbuf_tensor(f"ot{i}", [C, FREE], u8))
            for i in range(2)
        ]
        wg = [
            ctx.enter_context(nc.sbuf_tensor(f"wg{i}", [C, WG], f32))
            for i in range(2)
        ]
        v = ctx.enter_context(nc.sbuf_tensor("v", [C, FD], f32))
        h = ctx.enter_context(nc.sbuf_tensor("h", [C, WG], f32))
        bm1 = ctx.enter_context(nc.sbuf_tensor("bm1", [C, 1], f32))

        s_x = ctx.enter_context(nc.semaphore("s_x"))
        s_o = ctx.enter_context(nc.semaphore("s_o"))
        s_v = ctx.enter_context(nc.semaphore("s_v"))
        s_g = ctx.enter_context(nc.semaphore("s_g"))
        s_a = ctx.enter_context(nc.semaphore("s_a"))
        block = ctx.enter_context(nc.Block())

        # G slice i of the full-tile buffers / of the W-width buffers
        def gsl(buf, i):
            return buf[:, GOFF[i] : GOFF[i] + GS[i]]

        def wsl(buf, i):
            off = GOFF[i] - FD
            return buf[:, off : off + GS[i]]

        # semaphore value after instr #idx (1-based) of step t
        def va(t, idx):  # ACT: 7/step (SgD, SgG1, hG1, SgG2, hG2, SgG3, hG3)
            return 7 * t + idx

        def vv(t, idx):  # DVE: 3 preamble memsets, then 5/step
            return 3 + 5 * t + idx

        def vg(t, idx):  # GPSIMD: 3/step (TT-G1..3)
            return 3 * t + idx

        @block.sync
        def _(sync: bass.BassEngine):
            for t in range(T):
                if t >= NX:
                    # slot free once step t-NX consumed it
                    sync.wait_ge(s_v, vv(t - NX, 1))   # u'D read x
                    sync.wait_ge(s_g, vg(t - NX, 3))   # TT-G1..3 read x
                sync.dma_start(
                    out=xt[t % NX][:, :].rearrange("p (b f) -> p b f", b=BLOC),
                    in_=x_d[:, t].rearrange("b c f -> c b f"),
                ).then_inc(s_x, 16)
                if t >= 1:
                    sync.wait_ge(s_a, va(t - 1, 6))    # all Signs of t-1
                    sync.dma_start(
                        out=o_d[:, t - 1].rearrange("b c f -> c b f"),
                        in_=ot[(t - 1) % 2][:, :].rearrange(
                            "p (b f) -> p b f", b=BLOC
                        ),
                    ).then_inc(s_o, 16)
            sync.wait_ge(s_a, va(T - 1, 6))
            sync.dma_start(
                out=o_d[:, T - 1].rearrange("b c f -> c b f"),
                in_=ot[(T - 1) % 2][:, :].rearrange("p (b f) -> p b f", b=BLOC),
            ).then_inc(s_o, 16)
            sync.wait_ge(s_o, 16 * T)

        @block.vector
        def _(vector: bass.BassEngine):
            vector.memset(v[:, :], 0.0).then_inc(s_v, 1)
            vector.memset(h[:, :], 0.0).then_inc(s_v, 1)
            vector.memset(bm1[:, :], -1.0).then_inc(s_v, 1)
            for t in range(T):
                u = up[t % 2]
                o = ot[t % 2]
                w = wg[t % 2]
                # u'D = (v * 0.5) + x
                vector.wait_ge(s_x, 16 * (t + 1))
                if t >= 2:
                    vector.wait_ge(s_a, va(t - 2, 1))  # SignD(t-2) read u'D
                vector.scalar_tensor_tensor(
                    out=u[:, :FD], in0=v[:, :], scalar=0.5,
                    in1=xt[t % NX][:, :FD], op0=Alu.mult, op1=Alu.add,
                ).then_inc(s_v, 1)
                # wGi = (o * -1) + u'Gi
                for i in range(3):
                    vector.wait_ge(s_a, va(t, 2 * (i + 1)))  # SignGi(t)
                    vector.scalar_tensor_tensor(
                        out=wsl(w, i), in0=gsl(o, i), scalar=-1.0,
                        in1=gsl(u, i), op0=Alu.mult, op1=Alu.add,
                    ).then_inc(s_v, 1)
                # vD = (o * -1) + u'D   (SignD(t) already <= SignG3(t) wait)
                vector.scalar_tensor_tensor(
                    out=v[:, :], in0=o[:, :FD], scalar=-1.0, in1=u[:, :FD],
                    op0=Alu.mult, op1=Alu.add,
                ).then_inc(s_v, 1)

        @block.gpsimd
        def _(g: bass.BassEngine):
            for t in range(T):
                u = up[t % 2]
                g.wait_ge(s_x, 16 * (t + 1))
                for i in range(3):
                    if t >= 1:
                        g.wait_ge(s_a, va(t - 1, 2 * i + 3))  # hGi(t-1)
                    if t >= 2:
                        g.wait_ge(s_v, vv(t - 2, 2 + i))      # wGi(t-2)
                    g.tensor_tensor(
                        out=gsl(u, i), in0=wsl(h, i),
                        in1=gsl(xt[t % NX], i), op=Alu.add,
                    ).then_inc(s_g, 1)

        @block.scalar
        def _(scalar: bass.BassEngine):
            for t in range(T):
                u = up[t % 2]
                o = ot[t % 2]
                w = wg[t % 2]
                # SignD(t): o_u8 D-cols
                scalar.wait_ge(s_v, vv(t, 1))
                if t >= 2:
                    scalar.wait_ge(s_o, 16 * (t - 1))  # o slot stored
                scalar.activation(
                    out=o[:, :FD], in_=u[:, :FD],
                    func=Act.Sign, bias=bm1[:, :], scale=1.0,
                ).then_inc(s_a, 1)
                for i in range(3):
                    # SignGi(t)
                    scalar.wait_ge(s_g, vg(t, i + 1))
                    scalar.activation(
                        out=gsl(o, i), in_=gsl(u, i),
                        func=Act.Sign, bias=bm1[:, :], scale=1.0,
                    ).then_inc(s_a, 1)
                    # hGi(t) = Copy(wGi * 0.5)
                    scalar.wait_ge(s_v, vv(t, 2 + i))
                    scalar.activation(
                        out=wsl(h, i), in_=wsl(w, i),
                        func=Act.Copy, bias=0.0, scale=0.5,
                    ).then_inc(s_a, 1)

    return nc


def _get_nc():
    if "nc" not in _cached:
        _cached["nc"] = _build_nc()
    return _cached["nc"]


def kernel(x_seq: np.ndarray) -> np.ndarray:
    import os

    from concourse.bass_utils import run_bass_kernel_spmd

    x = np.ascontiguousarray(np.asarray(x_seq, dtype=np.float32)).reshape(
        B, T, C, HW
    )
    nc = _get_nc()
    in_maps = [{"x": x[i * BLOC : (i + 1) * BLOC]} for i in range(NCORES)]
    trace = bool(os.environ.get("LIF_TRACE"))
    out = run_bass_kernel_spmd(nc, in_maps, list(range(NCORES)), trace=trace)
    _cached["last_results"] = out
    o = np.concatenate([r["o"] for r in out.results], axis=0)
    return o.reshape(B, T, C, 32, 32).astype(np.float32)


# revision 4
# speedup vs baseline: 1.0464x; 1.0464x over previous
"""LIF (leaky integrate-and-fire) forward kernel for Trainium2, 8-core SPMD.

Reference semantics (per element, scan over T):
    u = 0.5*u + x_t
    o_t = (u - 1 >= 0) ? 1.0 : 0.0
    u = u - o_t

Sharding: pure data parallel over batch B=32 -> 4 batches per core.
Per-core shard: x [4, 16, 128, 1024] f32; C=128 on the SBUF partition dim,
(b, h*w) on the free dim -> a [128, 4096] tile per timestep.

Work is split between the PE (tensor) and DVE (vector) engines by column
range, with ACT (scalar) computing every spike threshold:

  P-region, cols [0, 2048), two 1024-wide sub-slices, state v_P in SBUF:
      u' = 0.5I @ v_P + I @ x_t      PE, fp32 matmuls into PSUM (exact:
                                     diagonal weights, zero partial sums)
      o  = Sign(u' - 1) -> uint8     ACT reading PSUM (sat cast: -1 -> 0)
      v  = (o * -1.0) + u'           DVE stt, in1 = PSUM, out SBUF

  D-region, cols [2048, 4096), state v_D in SBUF, all on DVE:
      u' = (v_D * 0.5) + x_t         stt
      o  = Sign(u' - 1) -> uint8     ACT
      v  = (o * -1.0) + u'           stt (u8 in0 casts for free)

All f32 arithmetic rounds identically to the jax reference; the only
divergence is u' == 1.0 exactly (Sign(0)=0 -> no spike where the
reference fires) - measure-zero on randn inputs, far inside the 2e-2
gate. PSUM usage: 2 x [128, 2048] f32 = all 8 banks, double-buffered
across steps. GPSIMD is left idle on purpose: concurrent GPSIMD and DVE
SBUF traffic halves DVE throughput (measured), making offload there a
net loss; PE||DVE shows no such contention (measured).

Engine loads per step: DVE ~6.7us, PE ~5.5us, ACT ~4.0us,
in-DMA 2 MiB (split into two 1 MiB transfers), out-DMA 0.5 MiB.
"""

import numpy as np

B, T, C, HW = 32, 16, 128, 1024
NCORES = 8
BLOC = B // NCORES  # 4 batches per core
FREE = BLOC * HW    # 4096
NX = 3              # x buffer slots per region
CP = 2048           # PE-integrated columns (2 sub-slices of 1024)
CD = FREE - CP      # DVE-integrated columns

_cached = {}


def _build_nc():
    import concourse.bass as bass
    import concourse.mybir as mybir
    from contextlib import ExitStack

    f32 = mybir.dt.float32
    u8 = mybir.dt.uint8
    Alu = mybir.AluOpType
    Act = mybir.ActivationFunctionType

    nc = bass.Bass()
    x_d = nc.declare_dram_parameter("x", [BLOC, T, C, HW], f32, isOutput=False)
    w5_d = nc.declare_dram_parameter("w5", [C, C], f32, isOutput=False)
    wi_d = nc.declare_dram_parameter("wi", [C, C], f32, isOutput=False)
    o_d = nc.declare_dram_parameter("o", [BLOC, T, C, HW], u8, isOutput=True)

    # batches 0-1 are the P region, 2-3 the D region (contiguous col halves)
    with ExitStack() as ctx:
        xp = [
            ctx.enter_context(nc.sbuf_tensor(f"xp{i}", [C, CP], f32))
            for i in range(NX)
        ]
        xd = [
            ctx.enter_context(nc.sbuf_tensor(f"xd{i}", [C, CD], f32))
            for i in range(NX)
        ]
        ud = [
            ctx.enter_context(nc.sbuf_tensor(f"ud{i}", [C, CD], f32))
            for i in range(2)
        ]
        ot = [
            ctx.enter_context(nc.sbuf_tensor(f"ot{i}", [C, FREE], u8))
            for i in range(2)
        ]
        vp = ctx.enter_context(nc.sbuf_tensor("vp", [C, CP], f32))
        vd = ctx.enter_context(nc.sbuf_tensor("vd", [C, CD], f32))
        w5 = ctx.enter_context(nc.sbuf_tensor("w5s", [C, C], f32))
        wi = ctx.enter_context(nc.sbuf_tensor("wis", [C, C], f32))
        bm1 = ctx.enter_context(nc.sbuf_tensor("bm1", [C, 1], f32))
        ps = [
            ctx.enter_context(nc.psum_tensor(f"ps{i}", [C, CP], f32))
            for i in range(2)
        ]

        s_x = ctx.enter_context(nc.semaphore("s_x"))
        s_w = ctx.enter_context(nc.semaphore("s_w"))
        s_o = ctx.enter_context(nc.semaphore("s_o"))
        s_v = ctx.enter_context(nc.semaphore("s_v"))
        s_a = ctx.enter_context(nc.semaphore("s_a"))
        s_pe = ctx.enter_context(nc.semaphore("s_pe"))
        block = ctx.enter_context(nc.Block())

        # semaphore value after instr #idx (1-based) of step t
        def va(t, i):  # ACT: 4/step (SignP1, SignD1, SignD2, SignP2)
            return 4 * t + i

        def vv(t, i):  # DVE: 3 memsets, then 5/step
            return 3 + 5 * t + i  # (u'D1, u'D2, resetP1, resetD12, resetP2)

        def vp_(t, i):  # PE: 8 matmuls/step
            return 8 * t + i

        @block.sync
        def _(sync: bass.BassEngine):
            sync.dma_start(out=w5[:, :], in_=w5_d[:, :]).then_inc(s_w, 16)
            sync.dma_start(out=wi[:, :], in_=wi_d[:, :]).then_inc(s_w, 16)
            for t in range(T):
                if t >= NX:
                    sync.wait_ge(s_pe, vp_(t - NX, 8))  # PE consumed xp
                sync.dma_start(
                    out=xp[t % NX][:, :].rearrange("p (b f) -> p b f", b=2),
                    in_=x_d[0:2, t].rearrange("b c f -> c b f"),
                ).then_inc(s_x, 16)
                if t >= NX:
                    sync.wait_ge(s_v, vv(t - NX, 2))    # u'D2 consumed xd
                sync.dma_start(
                    out=xd[t % NX][:, :].rearrange("p (b f) -> p b f", b=2),
                    in_=x_d[2:4, t].rearrange("b c f -> c b f"),
                ).then_inc(s_x, 16)
                if t >= 1:
                    sync.wait_ge(s_a, va(t - 1, 4))     # all Signs of t-1
                    sync.dma_start(
                        out=o_d[:, t - 1].rearrange("b c f -> c b f"),
                        in_=ot[(t - 1) % 2][:, :].rearrange(
                            "p (b f) -> p b f", b=BLOC
                        ),
                    ).then_inc(s_o, 16)
            sync.wait_ge(s_a, va(T - 1, 4))
            sync.dma_start(
                out=o_d[:, T - 1].rearrange("b c f -> c b f"),
                in_=ot[(T - 1) % 2][:, :].rearrange("p (b f) -> p b f", b=BLOC),
            ).then_inc(s_o, 16)
            sync.wait_ge(s_o, 16 * T)

        @block.tensor
        def _(pe: bass.BassEngine):
            pe.wait_ge(s_w, 32)
            for t in range(T):
                p = ps[t % 2]
                pe.wait_ge(s_x, 32 * t + 16)            # xp(t)
                if t >= 1:
                    pe.wait_ge(s_v, vv(t - 1, 5))       # v_P(t-1) final
                if t >= 2:
                    pe.wait_ge(s_a, va(t - 2, 4))       # psum slot free
                k = 0
                for sub in range(2):                    # 1024-wide sub-slices
                    lo = sub * 1024
                    for wmat, src in ((w5, vp), (wi, xp[t % NX])):
                        for b in range(2):              # 512-wide blocks
                            c0 = lo + b * 512
                            pe.matmul(
                                out=p[:, c0 : c0 + 512],
                                lhsT=wmat[:, :],
                                rhs=src[:, c0 : c0 + 512],
                                start=(wmat is w5),
                                stop=(wmat is wi),
                            ).then_inc(s_pe, 1)
                            k += 1

        @block.vector
        def _(vector: bass.BassEngine):
            vector.memset(vd[:, :], 0.0).then_inc(s_v, 1)
            vector.memset(vp[:, :], 0.0).then_inc(s_v, 1)
            vector.memset(bm1[:, :], -1.0).then_inc(s_v, 1)
            for t in range(T):
                p = ps[t % 2]
                o = ot[t % 2]
                u = ud[t % 2]
                # u'D1, u'D2 = (v_D * 0.5) + x_D   (1024-wide halves)
                vector.wait_ge(s_x, 32 * t + 32)        # xd(t)
                for i in range(2):
                    if t >= 2:
                        vector.wait_ge(s_a, va(t - 2, 2 + i))  # SignDi(t-2)
                    sl = slice(i * 1024, (i + 1) * 1024)
                    vector.scalar_tensor_tensor(
                        out=u[:, sl], in0=vd[:, sl], scalar=0.5,
                        in1=xd[t % NX][:, sl], op0=Alu.mult, op1=Alu.add,
                    ).then_inc(s_v, 1)
                # resetP1: v_P1 = (o_P1 * -1) + u'_P1(psum)
                vector.wait_ge(s_a, va(t, 1))           # SignP1(t)
                vector.scalar_tensor_tensor(
                    out=vp[:, :1024], in0=o[:, :1024], scalar=-1.0,
                    in1=p[:, :1024], op0=Alu.mult, op1=Alu.add,
                ).then_inc(s_v, 1)
                # resetD12: v_D = (o_D * -1) + u'_D
                vector.wait_ge(s_a, va(t, 3))           # SignD2(t)
                vector.scalar_tensor_tensor(
                    out=vd[:, :], in0=o[:, CP:], scalar=-1.0, in1=u[:, :],
                    op0=Alu.mult, op1=Alu.add,
                ).then_inc(s_v, 1)
                # resetP2
                vector.wait_ge(s_a, va(t, 4))           # SignP2(t)
                vector.scalar_tensor_tensor(
                    out=vp[:, 1024:], in0=o[:, 1024:2048], scalar=-1.0,
                    in1=p[:, 1024:], op0=Alu.mult, op1=Alu.add,
                ).then_inc(s_v, 1)

        @block.scalar
        def _(scalar: bass.BassEngine):
            for t in range(T):
                p = ps[t % 2]
                o = ot[t % 2]
                u = ud[t % 2]
                # SignP1
                scalar.wait_ge(s_pe, vp_(t, 4))
                if t >= 2:
                    scalar.wait_ge(s_o, 16 * (t - 1))   # o slot stored
                    scalar.wait_ge(s_v, vv(t - 2, 5))   # o slot read by DVE
                scalar.activation(
                    out=o[:, :1024], in_=p[:, :1024],
                    func=Act.Sign, bias=bm1[:, :], scale=1.0,
                ).then_inc(s_a, 1)
                # SignD1, SignD2
                for i in range(2):
                    scalar.wait_ge(s_v, vv(t, 1 + i))
                    sl = slice(CP + i * 1024, CP + (i + 1) * 1024)
                    usl = slice(i * 1024, (i + 1) * 1024)
                    scalar.activation(
                        out=o[:, sl], in_=u[:, usl],
                        func=Act.Sign, bias=bm1[:, :], scale=1.0,
                    ).then_inc(s_a, 1)
                # SignP2
                scalar.wait_ge(s_pe, vp_(t, 8))
                scalar.activation(
                    out=o[:, 1024:2048], in_=p[:, 1024:],
                    func=Act.Sign, bias=bm1[:, :], scale=1.0,
                ).then_inc(s_a, 1)

    return nc


def _get_nc():
    if "nc" not in _cached:
        _cached["nc"] = _build_nc()
    return _cached["nc"]


def kernel(x_seq: np.ndarray) -> np.ndarray:
    import os

    from concourse.bass_utils import run_bass_kernel_spmd

    x = np.ascontiguousarray(np.asarray(x_seq, dtype=np.float32)).reshape(
        B, T, C, HW
    )
    nc = _get_nc()
    w5 = (0.5 * np.eye(C)).astype(np.float32)
    wi = np.eye(C, dtype=np.float32)
    in_maps = [
        {"x": x[i * BLOC : (i + 1) * BLOC], "w5": w5, "wi": wi}
        for i in range(NCORES)
    ]
    trace = bool(os.environ.get("LIF_TRACE"))
    out = run_bass_kernel_spmd(nc, in_maps, list(range(NCORES)), trace=trace)
    _cached["last_results"] = out
    o = np.concatenate([r["o"] for r in out.results], axis=0)
    return o.reshape(B, T, C, 32, 32).astype(np.float32)


# revision 5
# speedup vs baseline: 1.7185x; 1.6423x over previous
"""LIF (leaky integrate-and-fire) forward kernel for Trainium2, 8-core SPMD.

Reference semantics (per element, scan over T):
    u = 0.5*u + x_t
    o_t = (u - 1 >= 0) ? 1.0 : 0.0
    u = u - o_t

Sharding: pure data parallel over batch B=32 -> 4 batches per core.
Per-core shard: x [4, 16, 128, 1024] f32; C=128 on the SBUF partition dim,
(b, h*w) on the free dim -> a [128, 4096] tile per timestep.

Per column-slice pipeline (NSL slices, state v per slice in SBUF):
    u' = (v * 0.5) + x_t         DVE scalar_tensor_tensor
    o  = Sign(u' - 1) -> uint8   ACT (saturating cast: -1 -> 0, so o=(u'>1);
                                 u'==1.0 exactly stores 0 where the ref
                                 fires - measure-zero on randn inputs)
    v  = (o * -1.0) + u'         DVE stt (u8 in0 casts at full speed)

Optionally the first CP columns' integrate runs on the PE instead
(psum = 0.5I @ v + I @ x, exact fp32 diagonal matmuls, 512-wide
sub-slices double-buffered in PSUM), with ACT reading PSUM for Sign and
DVE doing only the reset there. GPSIMD is left idle on purpose:
concurrent GPSIMD+DVE SBUF traffic halves DVE throughput (measured);
PE||DVE shows no contention (measured).

All f32 arithmetic rounds identically to the jax reference, so the
output is bit-exact away from the u'==1.0 boundary.
"""

import numpy as np

B, T, C, HW = 32, 16, 128, 1024
NCORES = 8
BLOC = B // NCORES  # 4 batches per core
FREE = BLOC * HW    # 4096
NX = 3              # x buffer slots

CP = 0              # PE-integrated columns (multiple of 512; 0 = no PE)
NPS = CP // 512     # PE sub-slices
CD = FREE - CP
NSL = 2             # DVE slices for the D region
WD = CD // NSL

_cached = {}


def _build_nc():
    import concourse.bass as bass
    import concourse.mybir as mybir
    from contextlib import ExitStack

    f32 = mybir.dt.float32
    u8 = mybir.dt.uint8
    Alu = mybir.AluOpType
    Act = mybir.ActivationFunctionType

    nc = bass.Bass()
    x_d = nc.declare_dram_parameter("x", [BLOC, T, C, HW], f32, isOutput=False)
    if CP:
        w5_d = nc.declare_dram_parameter("w5", [C, C], f32, isOutput=False)
        wi_d = nc.declare_dram_parameter("wi", [C, C], f32, isOutput=False)
    o_d = nc.declare_dram_parameter("o", [BLOC, T, C, HW], u8, isOutput=True)

    # DVE: per step, NSL integrate stts + NSL+NPS reset stts
    NV = 2 * NSL + NPS
    # ACT: per step, NPS psum Signs then NSL sbuf Signs
    NA = NPS + NSL

    with ExitStack() as ctx:
        xt = [
            ctx.enter_context(nc.sbuf_tensor(f"xt{i}", [C, FREE], f32))
            for i in range(NX)
        ]
        ud = [
            ctx.enter_context(nc.sbuf_tensor(f"ud{i}", [C, CD], f32))
            for i in range(2)
        ]
        ot = [
            ctx.enter_context(nc.sbuf_tensor(f"ot{i}", [C, FREE], u8))
            for i in range(2)
        ]
        vd = ctx.enter_context(nc.sbuf_tensor("vd", [C, CD], f32))
        bm1 = ctx.enter_context(nc.sbuf_tensor("bm1", [C, 1], f32))
        if CP:
            vp = ctx.enter_context(nc.sbuf_tensor("vp", [C, CP], f32))
            w5 = ctx.enter_context(nc.sbuf_tensor("w5s", [C, C], f32))
            wi = ctx.enter_context(nc.sbuf_tensor("wis", [C, C], f32))
            ps = [
                ctx.enter_context(nc.psum_tensor(f"ps{i}", [C, CP], f32))
                for i in range(2)
            ]

        s_x = ctx.enter_context(nc.semaphore("s_x"))
        s_o = ctx.enter_context(nc.semaphore("s_o"))
        s_v = ctx.enter_context(nc.semaphore("s_v"))
        s_a = ctx.enter_context(nc.semaphore("s_a"))
        if CP:
            s_w = ctx.enter_context(nc.semaphore("s_w"))
            s_pe = ctx.enter_context(nc.semaphore("s_pe"))
        block = ctx.enter_context(nc.Block())

        def va(t, i):  # ACT counter after instr i (1-based) of step t
            return NA * t + i

        def vv(t, i):  # DVE counter (2 memset preamble + NV/step)
            return (2 + (1 if CP else 0)) + NV * t + i

        def vpe(t, i):  # PE counter: 4 matmul calls per sub-slice
            return 4 * NPS * t + i

        # D-slice j columns within the D region / within the full tile
        def dsl(buf, j):
            return buf[:, j * WD : (j + 1) * WD]

        def dslf(buf, j):
            return buf[:, CP + j * WD : CP + (j + 1) * WD]

        @block.sync
        def _(sync: bass.BassEngine):
            if CP:
                sync.dma_start(out=w5[:, :], in_=w5_d[:, :]).then_inc(s_w, 16)
                sync.dma_start(out=wi[:, :], in_=wi_d[:, :]).then_inc(s_w, 16)
            for t in range(T):
                if t >= NX:
                    sync.wait_ge(s_v, vv(t - NX, NV))   # integrates read x
                    if CP:
                        sync.wait_ge(s_pe, vpe(t - NX, 4 * NPS))
                sync.dma_start(
                    out=xt[t % NX][:, :].rearrange("p (b f) -> p b f", b=BLOC),
                    in_=x_d[:, t].rearrange("b c f -> c b f"),
                ).then_inc(s_x, 16)
                if t >= 1:
                    sync.wait_ge(s_a, va(t - 1, NA))    # all Signs of t-1
                    sync.dma_start(
                        out=o_d[:, t - 1].rearrange("b c f -> c b f"),
                        in_=ot[(t - 1) % 2][:, :].rearrange(
                            "p (b f) -> p b f", b=BLOC
                        ),
                    ).then_inc(s_o, 16)
            sync.wait_ge(s_a, va(T - 1, NA))
            sync.dma_start(
                out=o_d[:, T - 1].rearrange("b c f -> c b f"),
                in_=ot[(T - 1) % 2][:, :].rearrange("p (b f) -> p b f", b=BLOC),
            ).then_inc(s_o, 16)
            sync.wait_ge(s_o, 16 * T)

        if CP:

            @block.tensor
            def _(pe: bass.BassEngine):
                pe.wait_ge(s_w, 32)
                for t in range(T):
                    p = ps[t % 2]
                    pe.wait_ge(s_x, 16 * (t + 1))
                    if t >= 2:
                        pe.wait_ge(s_a, va(t - 2, NPS))  # psum Signs done
                    for s in range(NPS):
                        c0 = s * 512
                        if t >= 1:
                            # v_P sub written by resetPs(t-1)
                            pe.wait_ge(s_v, vv(t - 1, 2 * NSL + s + 1))
                        for wmat, src in ((w5, vp), (wi, xt[t % NX])):
                            pe.matmul(
                                out=p[:, c0 : c0 + 512],
                                lhsT=wmat[:, :],
                                rhs=src[:, c0 : c0 + 512],
                                start=(wmat is w5),
                                stop=(wmat is wi),
                            ).then_inc(s_pe, 2)

        @block.vector
        def _(vector: bass.BassEngine):
            vector.memset(vd[:, :], 0.0).then_inc(s_v, 1)
            vector.memset(bm1[:, :], -1.0).then_inc(s_v, 1)
            if CP:
                vector.memset(vp[:, :], 0.0).then_inc(s_v, 1)
            for t in range(T):
                o = ot[t % 2]
                u = ud[t % 2]
                vector.wait_ge(s_x, 16 * (t + 1))
                # u'Dj = (v_Dj * 0.5) + x_Dj
                for j in range(NSL):
                    if t >= 2:
                        vector.wait_ge(s_a, va(t - 2, NPS + 1 + j))
                    vector.scalar_tensor_tensor(
                        out=dsl(u, j), in0=dsl(vd, j), scalar=0.5,
                        in1=dslf(xt[t % NX], j), op0=Alu.mult, op1=Alu.add,
                    ).then_inc(s_v, 1)
                # resetPs: v_Ps = (o_Ps * -1) + psum_Ps
                for s in range(NPS):
                    vector.wait_ge(s_a, va(t, s + 1))
                    c0 = s * 512
                    vector.scalar_tensor_tensor(
                        out=vp[:, c0 : c0 + 512], in0=o[:, c0 : c0 + 512],
                        scalar=-1.0, in1=ps[t % 2][:, c0 : c0 + 512],
                        op0=Alu.mult, op1=Alu.add,
                    ).then_inc(s_v, 1)
                # resetDj: v_Dj = (o_Dj * -1) + u'Dj
                for j in range(NSL):
                    vector.wait_ge(s_a, va(t, NPS + 1 + j))
                    vector.scalar_tensor_tensor(
                        out=dsl(vd, j), in0=dslf(o, j), scalar=-1.0,
                        in1=dsl(u, j), op0=Alu.mult, op1=Alu.add,
                    ).then_inc(s_v, 1)

        @block.scalar
        def _(scalar: bass.BassEngine):
            for t in range(T):
                o = ot[t % 2]
                u = ud[t % 2]
                first = True
                # SignPs from PSUM
                for s in range(NPS):
                    scalar.wait_ge(s_pe, vpe(t, 4 * (s + 1)))
                    if first and t >= 2:
                        scalar.wait_ge(s_o, 16 * (t - 1))
                        scalar.wait_ge(s_v, vv(t - 2, NV))
                    first = False
                    c0 = s * 512
                    scalar.activation(
                        out=o[:, c0 : c0 + 512], in_=ps[t % 2][:, c0 : c0 + 512],
                        func=Act.Sign, bias=bm1[:, :], scale=1.0,
                    ).then_inc(s_a, 1)
                # SignDj
                for j in range(NSL):
                    scalar.wait_ge(s_v, vv(t, 1 + j))
                    if first and t >= 2:
                        scalar.wait_ge(s_o, 16 * (t - 1))
                        scalar.wait_ge(s_v, vv(t - 2, NV))
                    first = False
                    scalar.activation(
                        out=dslf(o, j), in_=dsl(u, j),
                        func=Act.Sign, bias=bm1[:, :], scale=1.0,
                    ).then_inc(s_a, 1)

    return nc


def _get_nc():
    if "nc" not in _cached:
        _cached["nc"] = _build_nc()
    return _cached["nc"]


def kernel(x_seq: np.ndarray) -> np.ndarray:
    import os

    from concourse.bass_utils import run_bass_kernel_spmd

    x = np.ascontiguousarray(np.asarray(x_seq, dtype=np.float32)).reshape(
        B, T, C, HW
    )
    nc = _get_nc()
    in_maps = []
    for i in range(NCORES):
        m = {"x": x[i * BLOC : (i + 1) * BLOC]}
        if CP:
            m["w5"] = (0.5 * np.eye(C)).astype(np.float32)
            m["wi"] = np.eye(C, dtype=np.float32)
        in_maps.append(m)
    trace = bool(os.environ.get("LIF_TRACE"))
    out = run_bass_kernel_spmd(nc, in_maps, list(range(NCORES)), trace=trace)
    _cached["last_results"] = out
    o = np.concatenate([r["o"] for r in out.results], axis=0)
    return o.reshape(B, T, C, 32, 32).astype(np.float32)
